# revision 12
# baseline (speedup 1.0000x reference)
"""Trainium2 Bass kernel for nn_ContrastiveLoss (segment_reduce).

Strategy (data-parallel over batch, 2 samples per core on 8 cores):
  - Host: transpose emb_q [B,C,H,W] -> pixel-major [B, HW, C] (pure layout),
    labels -> int32.  Shard by batch: core i gets samples [2i, 2i+1].
  - Device per core, per sample: stream pixel tiles [128, G*19]; compute
    per-pixel L2 norm (ACT square + DVE grouped reduce + ACT sqrt + DVE
    reciprocal); normalized features zn[p,u,0:19] = eq*w, zn[p,u,19] = 1;
    one-hot mask[p,u,k] = (label==k) via DVE is_equal against an iota tile.
    Segment-reduce: for each 128-pixel chunk u, PE matmul
    acc[19,20] += mask_u.T @ zn_u  accumulated in PSUM over the whole sample.
    acc[k, 0:19] = sum of normalized features of class k; acc[k,19] = count.
  - Host: per-sample means -> logits vs normalized emb_k -> log_softmax ->
    masked CE mean -> scalar loss (exact tiny math in numpy f32).
"""

import os
import numpy as np

import concourse.bass as bass
import concourse.mybir as mybir
import concourse.tile as tile
from concourse.bass_utils import run_bass_kernel_spmd

# ---------------------------------------------------------------- constants
N_CLASSES = 19
K = N_CLASSES + 1          # 20: 19 classes + counts column
TAU = 0.1
B, C, H, W = 16, 19, 512, 512
HW = H * W                 # 262144
NCORES = 8
SPC = B // NCORES          # samples per core = 2
P = 128                    # partitions / pixels per matmul chunk
G = 128                    # chunks per tile -> tile covers P*G = 16384 pixels
T = HW // (P * G)          # tiles per sample = 16
F32 = mybir.dt.float32
I32 = mybir.dt.int32

# ----------------------------------------------------- sync-wait splitting
# The walrus build in this container rejects instructions carrying more than
# ONE sync wait ("Too many sync wait commands").  Tile's scheduler freely
# attaches several waits to one instruction.  Post-process the BIR: move
# excess waits onto same-engine NOPs inserted immediately before.
def _split_sync_waits(nc, maxw=1):
    for f in nc.m.functions:
        for bb in f.blocks:
            newl = []
            changed = False
            for ins in bb.instructions:
                si = ins.sync_info
                w = list(si.on_wait) if si is not None else []
                if len(w) > maxw:
                    extra = w[:-maxw]
                    for j in range(0, len(extra), maxw):
                        grp = extra[j : j + maxw]
                        nop = mybir.InstNoOp(
                            name=f"{ins.name}_wsplit{j}", ins=[], outs=[]
                        )
                        nop.engine = ins.engine
                        nop.sync_info = mybir.SyncInfo(on_wait=grp, on_update=[])
                        newl.append(nop)
                    ins.sync_info = mybir.SyncInfo(
                        on_wait=w[-maxw:], on_update=list(si.on_update)
                    )
                    changed = True
                newl.append(ins)
            if changed:
                bb.instructions = newl


# ------------------------------------------------------------ device kernel
def _build_nc():
    nc = bass.Bass()
    eqt = nc.dram_tensor("eqt", [SPC * HW, C], F32, kind="ExternalInput")
    lab = nc.dram_tensor("lab", [SPC * HW, 1], I32, kind="ExternalInput")
    iota = nc.dram_tensor("iota", [P, N_CLASSES], F32, kind="ExternalInput")
    # 4 PE column-group strips accumulated separately; host sums them
    out = nc.dram_tensor("out", [SPC, 4, N_CLASSES, K], F32, kind="ExternalOutput")

    # pixel assignment: pixel index = ((s*T + t)*P + p)*G + g
    eqt_v = eqt[:, :].rearrange("(s t p g) c -> s t p (g c)", s=SPC, t=T, p=P, g=G)
    lab_v = lab[:, :].rearrange("(s t p g) o -> s t p (g o)", s=SPC, t=T, p=P, g=G)

    with tile.TileContext(nc) as tc:
        with (
            tc.tile_pool(name="const", bufs=1) as cpool,
            tc.tile_pool(name="sbuf", bufs=2) as pool,
            tc.tile_pool(name="small", bufs=3) as spool,
            tc.tile_pool(name="psum", bufs=2, space="PSUM") as ppool,
            tc.tile_pool(name="res", bufs=2) as rpool,
        ):
            iota_t = cpool.tile([P, N_CLASSES], F32)
            nc.sync.dma_start(iota_t[:], iota[:, :])

            for s in range(SPC):
                acc = ppool.tile([P, K], F32)
                for t_ in range(T):
                    eqt_t = pool.tile([P, G * C], F32, tag="eqt")
                    lab_t = spool.tile([P, G], I32, tag="lab")
                    nc.sync.dma_start(eqt_t[:], eqt_v[s, t_])
                    nc.sync.dma_start(lab_t[:], lab_v[s, t_])

                    eqt3 = eqt_t[:].rearrange("p (g c) -> p g c", c=C)

                    # labels -> f32 (GPSIMD: 1-input ops run near line-rate)
                    labf = spool.tile([P, G], F32, tag="labf")
                    nc.gpsimd.tensor_copy(labf[:], lab_t[:])

                    # per-pixel squared norm over the 19 channels (square on ACT)
                    sq = pool.tile([P, G * C], F32, tag="sq")
                    nc.scalar.activation(
                        sq[:], eqt_t[:], mybir.ActivationFunctionType.Square
                    )
                    n2 = spool.tile([P, G], F32, tag="n2")
                    nc.vector.reduce_sum(
                        n2[:], sq[:].rearrange("p (g c) -> p g c", c=C),
                        axis=mybir.AxisListType.X,
                    )
                    # norm = sqrt(max(n2, eps)); w = 1/norm
                    nc.gpsimd.tensor_scalar_max(n2[:], n2[:], 1e-24)
                    nrm = spool.tile([P, G], F32, tag="nrm")
                    nc.scalar.activation(
                        nrm[:], n2[:], mybir.ActivationFunctionType.Sqrt
                    )
                    wrec = spool.tile([P, G], F32, tag="wrec")
                    nc.vector.reciprocal(wrec[:], nrm[:])

                    # zn[p,g,0:19] = eq * w (GPSIMD mult frees DVE); zn[p,g,19] = 1.0
                    zn = pool.tile([P, G * K], F32, tag="zn")
                    zn3 = zn[:].rearrange("p (g k) -> p g k", k=K)
                    nc.gpsimd.tensor_tensor(
                        out=zn3[:, :, 0:C],
                        in0=eqt3,
                        in1=wrec[:, :, None].to_broadcast([P, G, C]),
                        op=mybir.AluOpType.mult,
                    )
                    nc.gpsimd.memset(zn3[:, :, C : C + 1], 1.0)

                    # mask[p,g,k] = (label[p,g] == k) (is_equal only exists on DVE)
                    mask = pool.tile([P, G * N_CLASSES], F32, tag="mask")
                    mask3 = mask[:].rearrange("p (g k) -> p g k", k=N_CLASSES)
                    nc.vector.tensor_tensor(
                        out=mask3,
                        in0=labf[:, :, None].to_broadcast([P, G, N_CLASSES]),
                        in1=iota_t[:, None, :].to_broadcast([P, G, N_CLASSES]),
                        op=mybir.AluOpType.is_equal,
                    )

                    # segment-reduce 128 pixels per chunk into PSUM; 4 chunks
                    # run concurrently in distinct 32-wide PE column groups
                    for u in range(G):
                        j = u % 4
                        nc.tensor.matmul(
                            out=acc[32 * j : 32 * j + N_CLASSES, :],
                            lhsT=mask3[:, u, :],
                            rhs=zn3[:, u, :],
                            start=(t_ == 0 and u == j),
                            stop=(t_ == T - 1 and u == G - 4 + j),
                            tile_position=(0, 32 * j),
                            skip_group_check=True,
                        )

                res = rpool.tile([P, K], F32)
                nc.vector.tensor_copy(res[:], acc[:])
                for j in range(4):
                    nc.sync.dma_start(
                        out[s, j, :, :], res[32 * j : 32 * j + N_CLASSES, :]
                    )

    _split_sync_waits(nc)
    return nc


_NC = None
LAST_RESULTS = None


def _get_nc():
    global _NC
    if _NC is None:
        _NC = _build_nc()
    return _NC


# --------------------------------------------------------------- host entry
def _make_in_maps(inputs):
    emb_q = np.asarray(inputs["emb_q"], dtype=np.float32)
    labels_np = np.asarray(inputs["labels"])

    # pixel-major layout + int32 labels (255 fits losslessly)
    eqt_full = np.ascontiguousarray(
        emb_q.transpose(0, 2, 3, 1).reshape(B, HW, C)
    )
    lab_full = np.ascontiguousarray(labels_np.reshape(B, HW).astype(np.int32))
    iota_np = np.ascontiguousarray(
        np.broadcast_to(np.arange(N_CLASSES, dtype=np.float32), (P, N_CLASSES))
    )

    in_maps = []
    for i in range(NCORES):
        in_maps.append(
            {
                "eqt": eqt_full[i * SPC : (i + 1) * SPC].reshape(SPC * HW, C),
                "lab": lab_full[i * SPC : (i + 1) * SPC].reshape(SPC * HW, 1),
                "iota": iota_np,
            }
        )
    return in_maps


def kernel(emb_k, emb_q, labels, epoch):
    emb_k = np.asarray(emb_k, dtype=np.float32)
    epoch_val = int(np.asarray(epoch))
    in_maps = _make_in_maps({"emb_q": emb_q, "labels": labels})

    nc = _get_nc()
    res = run_bass_kernel_spmd(
        nc,
        in_maps,
        core_ids=list(range(NCORES)),
        trace=bool(int(os.environ.get("KERNEL_TRACE", "0"))),
    )
    global LAST_RESULTS
    LAST_RESULTS = res

    # [16, 4, 19, 20] -> sum strips -> per-sample sums (cols 0:19), counts (col 19)
    outs = np.concatenate([r["out"] for r in res.results], axis=0).sum(axis=1)
    sums = outs[:, :, :N_CLASSES].astype(np.float32)
    counts = outs[:, :, N_CLASSES].astype(np.float32)

    # tiny CE epilogue in f32, mirroring the reference
    ekn = emb_k / np.maximum(
        np.linalg.norm(emb_k, axis=-1, keepdims=True), 1e-12
    ).astype(np.float32)
    means = sums / np.maximum(counts, 1.0)[:, :, None]          # [B, 19, 19]
    logits = np.einsum("bkc,nc->bkn", means, ekn).astype(np.float32) / np.float32(TAU)
    m = logits.max(axis=-1, keepdims=True)
    shifted = logits - m
    logp = shifted - np.log(np.exp(shifted).sum(axis=-1, keepdims=True))
    ce = -np.einsum("bkk->bk", logp)                            # diag, [B, 19]
    valid = counts > 0.0
    nvalid = valid.sum(axis=-1).astype(np.float32)
    per_sample = (ce * valid).sum(axis=-1) / np.maximum(nvalid, 1.0)
    total = np.where(nvalid > 0, per_sample, 0.0).sum() / np.float32(B)
    result = np.float32(total) if epoch_val != 0 else np.float32(0.0)
    return np.asarray(result, dtype=np.float32)
